# revision 1
# baseline (speedup 1.0000x reference)
"""AttentionMIL Trainium2 kernel.

Math (per bag of 512 instances):
    emb    = relu(x @ w_enc + b_enc)            [512, 128]
    a      = tanh(emb @ w_att + b_att)          [512, 64]
    logits = a @ w_score (+ b_score, dropped: softmax shift-invariant)
    attn   = softmax(logits) within the bag
    bag    = sum_i attn[i] * emb[i]             [128]
    score  = bag @ w_cls + b_cls                [2]

Distribution: data-parallel over bags. 8 NeuronCores, 8 bags (4096
instances) per core, weights replicated, no cross-core communication.
Each core returns its 8 bags' scores transposed [2, 8]; host stacks.

Layout: the host pre-transposes each core's x shard to x^T [1024, 4096]
and converts it (and the mat-mul weights) to bf16, halving the HBM
traffic — the kernel is DMA-bound — and putting the contraction dim
(d_in) on SBUF partitions directly, avoiding 256 on-chip PE transposes
+ PSUM evacuation per core. Matmuls accumulate in f32 PSUM; biases,
softmax and the bag reduction stay f32 (measured ~7e-4 rel err vs the
f32 reference). Everything on-chip stays transposed (emb^T [128 emb,
512 inst], a^T [64 att, 512 inst], logits [1, 512]) so per-partition
bias APs implement the +b terms and the per-bag softmax is a free-axis
reduce. The softmax skips the max-shift (logits = a @ w_score with a in
(-1,1) tanh-bounded, so exp cannot overflow) and defers 1/denominator
to the [2, 8] classifier epilogue. The bag-weighted sum multiplies
emb^T by the exp row broadcast across partitions via a K=1 matmul with
a ones column. Per-bag emission is software-pipelined (bag b's softmax
tail is emitted after bag b+1's encoder matmuls) so the in-order PE
queue never head-of-line blocks; steady state runs at the ~360 GB/s
HBM roofline (~2.9 us per 1.05 MB bag slab).
"""

import sys

sys.path.insert(0, "/opt/trn_rl_repo")

import numpy as np

N_INST = 32768
N_BAGS = 64
D_IN = 1024
D_EMB = 128
D_ATT = 64
N_CLS = 2

N_CORES = 8
BAGS_PER_CORE = N_BAGS // N_CORES          # 8
INST_PER_BAG = N_INST // N_BAGS            # 512
INST_PER_CORE = N_INST // N_CORES          # 4096
DIN_CHUNKS = D_IN // 128                   # 8
SLAB_SPLIT = 2                             # DMAs per bag slab
CH_PER_DMA = DIN_CHUNKS // SLAB_SPLIT      # 2

_CACHE = {}


def _build():
    import concourse.bacc as bacc
    import concourse.mybir as mybir
    import concourse.tile as tile

    f32 = mybir.dt.float32
    f32r = mybir.dt.float32r
    bf16 = mybir.dt.bfloat16
    AF = mybir.ActivationFunctionType

    nc = bacc.Bacc("TRN2", target_bir_lowering=False, debug=False,
                   enable_asserts=False, num_devices=N_CORES)

    xt = nc.dram_tensor("xt", [D_IN, INST_PER_CORE], bf16, kind="ExternalInput")
    w_enc = nc.dram_tensor("w_enc", [128, D_IN], bf16, kind="ExternalInput")
    b_enc = nc.dram_tensor("b_enc", [D_EMB], f32, kind="ExternalInput")
    w_att = nc.dram_tensor("w_att", [D_EMB, D_ATT], bf16, kind="ExternalInput")
    b_att = nc.dram_tensor("b_att", [D_ATT], f32, kind="ExternalInput")
    w_score = nc.dram_tensor("w_score", [D_ATT], bf16, kind="ExternalInput")
    w_cls = nc.dram_tensor("w_cls", [D_EMB, N_CLS], f32, kind="ExternalInput")
    b_cls = nc.dram_tensor("b_cls", [N_CLS], f32, kind="ExternalInput")
    out = nc.dram_tensor("out", [N_CLS, BAGS_PER_CORE], f32,
                         kind="ExternalOutput")

    with tile.TileContext(nc) as tc:
        with (
            tc.tile_pool(name="const", bufs=1) as const,
            tc.tile_pool(name="xt", bufs=6) as xt_pool,
            tc.tile_pool(name="work", bufs=3) as work,
            tc.tile_pool(name="ps", bufs=2, space="PSUM") as ps,
        ):
            # ---- replicated weights ----
            # host supplies w_enc pre-packed as [128 din-part, chunk*emb]
            wenc_sb = const.tile([128, DIN_CHUNKS, D_EMB], bf16)
            nc.sync.dma_start(
                out=wenc_sb,
                in_=w_enc[:, :].rearrange("p (c e) -> p c e", c=DIN_CHUNKS),
            )
            benc_sb = const.tile([D_EMB, 1], f32)
            nc.scalar.dma_start(
                out=benc_sb, in_=b_enc[:].rearrange("(p one) -> p one", one=1))
            watt_sb = const.tile([D_EMB, D_ATT], bf16)
            nc.scalar.dma_start(out=watt_sb, in_=w_att[:, :])
            batt_sb = const.tile([D_ATT, 1], f32)
            nc.scalar.dma_start(
                out=batt_sb, in_=b_att[:].rearrange("(p one) -> p one", one=1))
            wscore_sb = const.tile([D_ATT, 1], bf16)
            nc.scalar.dma_start(
                out=wscore_sb,
                in_=w_score[:].rearrange("(p one) -> p one", one=1))
            wcls_sb = const.tile([D_EMB, N_CLS], f32)
            nc.scalar.dma_start(out=wcls_sb, in_=w_cls[:, :])
            bcls_sb = const.tile([N_CLS, 1], f32)
            nc.scalar.dma_start(
                out=bcls_sb, in_=b_cls[:].rearrange("(p one) -> p one", one=1))
            ones_row = const.tile([1, 128], bf16)
            nc.vector.memset(ones_row, 1.0)
            ones_f32r = const.tile([1, N_CLS], f32r)
            ones_tmp = const.tile([1, N_CLS], f32)
            nc.vector.memset(ones_tmp, 1.0)
            nc.vector.tensor_copy(ones_f32r, ones_tmp)

            # unnormalized bag embeddings (columns) + softmax denominators
            bag_all = const.tile([D_EMB, BAGS_PER_CORE], f32)
            den_all = const.tile([1, BAGS_PER_CORE], f32)

            xt_re = xt[:, :].rearrange("(c p) i -> p c i", p=128)

            def emit_enc(b):
                i0 = b * INST_PER_BAG
                # split the bag slab into DMAs so the first encoder
                # matmuls start as soon as their chunks land
                parts = []
                for j in range(SLAB_SPLIT):
                    part = xt_pool.tile([128, CH_PER_DMA, INST_PER_BAG], bf16,
                                        tag=f"slab{j}")
                    c0 = j * CH_PER_DMA
                    nc.sync.dma_start(
                        out=part,
                        in_=xt_re[:, c0:c0 + CH_PER_DMA,
                                  i0:i0 + INST_PER_BAG])
                    parts.append(part)

                # emb^T = relu(sum_c w_enc_c.T @ xt_c + b_enc)
                ps_emb = ps.tile([D_EMB, INST_PER_BAG], f32, tag="emb")
                for c in range(DIN_CHUNKS):
                    nc.tensor.matmul(ps_emb[:, :], wenc_sb[:, c, :],
                                     parts[c // CH_PER_DMA][:, c % CH_PER_DMA, :],
                                     start=(c == 0), stop=(c == DIN_CHUNKS - 1))
                embT = work.tile([D_EMB, INST_PER_BAG], bf16, tag="embT")
                nc.scalar.activation(embT, ps_emb, AF.Relu, bias=benc_sb,
                                     scale=1.0)
                return embT

            def emit_tail(b, embT):
                # a^T = tanh(w_att.T @ emb^T + b_att)
                ps_a = ps.tile([D_ATT, INST_PER_BAG], f32, tag="a")
                nc.tensor.matmul(ps_a[:, :], watt_sb[:, :], embT[:, :],
                                 start=True, stop=True)
                aT = work.tile([D_ATT, INST_PER_BAG], bf16, tag="aT")
                nc.scalar.activation(aT, ps_a, AF.Tanh, bias=batt_sb, scale=1.0)

                # logits = w_score.T @ a^T   [1, 512]
                ps_l = ps.tile([1, INST_PER_BAG], f32, tag="logit")
                nc.tensor.matmul(ps_l[:, :], wscore_sb[:, :], aT[:, :],
                                 start=True, stop=True)

                # softmax numerator row + denominator (normalization
                # deferred). No max-shift: logits = a @ w_score with
                # a in (-1,1), so |logits| <= ||w_score||_1 ~ 6 — exp is safe.
                e_row = work.tile([1, INST_PER_BAG], bf16, tag="e_row")
                nc.scalar.activation(e_row, ps_l, AF.Exp, scale=1.0)
                nc.vector.reduce_sum(den_all[0:1, b:b + 1], e_row,
                                     axis=mybir.AxisListType.X)

                # broadcast e row across 128 partitions via K=1 matmul
                ps_bc = ps.tile([D_EMB, INST_PER_BAG], f32, tag="bc")
                nc.tensor.matmul(ps_bc[:, :], ones_row[:, :], e_row[:, :],
                                 start=True, stop=True)

                # unnormalized bag = sum_i emb^T[:, i] * e[i]
                scratch = work.tile([D_EMB, INST_PER_BAG], bf16, tag="scratch")
                nc.vector.tensor_mul(scratch, embT[:, :], ps_bc[:, :])
                nc.vector.reduce_sum(bag_all[:, b:b + 1], scratch,
                                     axis=mybir.AxisListType.X)

            # software pipeline: emit bag b's dependent tail after bag b+1's
            # encoder matmuls so the in-order PE queue never head-of-line
            # blocks on the softmax chain
            def emit_tail_halves(b, embT):
                # the last bag's tail is the serial end-of-kernel chain:
                # split it into two 256-instance halves so the PE/ACT/DVE
                # stages pipeline against each other
                H = INST_PER_BAG // 2
                den_h = work.tile([1, 2], f32, tag="den_h")
                bag_h = work.tile([D_EMB, 2], f32, tag="bag_h")
                for h in range(2):
                    sl = slice(h * H, (h + 1) * H)
                    ps_a = ps.tile([D_ATT, H], f32, tag="a")
                    nc.tensor.matmul(ps_a[:, :], watt_sb[:, :], embT[:, sl],
                                     start=True, stop=True)
                    aT = work.tile([D_ATT, H], bf16, tag="aT")
                    nc.scalar.activation(aT, ps_a, AF.Tanh, bias=batt_sb,
                                         scale=1.0)
                    ps_l = ps.tile([1, H], f32, tag="logit")
                    nc.tensor.matmul(ps_l[:, :], wscore_sb[:, :], aT[:, :],
                                     start=True, stop=True)
                    e_row = work.tile([1, H], bf16, tag="e_row")
                    nc.scalar.activation(e_row, ps_l, AF.Exp, scale=1.0)
                    nc.vector.reduce_sum(den_h[0:1, h:h + 1], e_row,
                                         axis=mybir.AxisListType.X)
                    ps_bc = ps.tile([D_EMB, H], f32, tag="bc")
                    nc.tensor.matmul(ps_bc[:, :], ones_row[:, :], e_row[:, :],
                                     start=True, stop=True)
                    scratch = work.tile([D_EMB, H], bf16, tag="scratch")
                    nc.vector.tensor_mul(scratch, embT[:, sl], ps_bc[:, :])
                    nc.vector.reduce_sum(bag_h[:, h:h + 1], scratch,
                                         axis=mybir.AxisListType.X)
                nc.vector.tensor_add(den_all[0:1, b:b + 1], den_h[0:1, 0:1],
                                     den_h[0:1, 1:2])
                nc.vector.tensor_add(bag_all[:, b:b + 1], bag_h[:, 0:1],
                                     bag_h[:, 1:2])

            prev = None
            for b in range(BAGS_PER_CORE):
                embT = emit_enc(b)
                if prev is not None:
                    emit_tail(b - 1, prev)
                prev = embT
            emit_tail_halves(BAGS_PER_CORE - 1, prev)

            # scores^T = (w_cls.T @ bag_u) * (1/den) + b_cls   [2, 8]
            ps_s = ps.tile([N_CLS, BAGS_PER_CORE], f32, tag="logit")
            nc.tensor.matmul(ps_s[:, :], wcls_sb[:, :], bag_all[:, :],
                             start=True, stop=True)
            rden_row = const.tile([1, BAGS_PER_CORE], f32r)
            with nc.allow_low_precision(reason="1/denom at f32r, ~1e-4 rel"):
                nc.vector.reciprocal(rden_row, den_all)
            ps_r = ps.tile([N_CLS, BAGS_PER_CORE], f32, tag="bc")
            nc.tensor.matmul(ps_r[:, :], ones_f32r[:, :], rden_row[:, :],
                             start=True, stop=True)
            s_u = const.tile([N_CLS, BAGS_PER_CORE], f32)
            nc.scalar.activation(s_u, ps_s[:, :], AF.Copy)
            s_n = const.tile([N_CLS, BAGS_PER_CORE], f32)
            nc.vector.tensor_mul(s_n, s_u, ps_r[:, :])
            scores = const.tile([N_CLS, BAGS_PER_CORE], f32)
            nc.scalar.activation(scores, s_n, AF.Identity, bias=bcls_sb,
                                 scale=1.0)
            nc.scalar.dma_start(out=out[:, :], in_=scores)

    nc.compile()
    return nc


def _numpy_fallback(x, seg, w_enc, b_enc, w_att, b_att, w_score, b_score,
                    w_cls, b_cls):
    emb = np.maximum(x @ w_enc + b_enc, 0.0)
    a = np.tanh(emb @ w_att + b_att)
    logits = a @ w_score + b_score[0]
    out = np.zeros((N_BAGS, N_CLS), dtype=np.float32)
    for bag in range(N_BAGS):
        mask = seg == bag
        lg = logits[mask]
        e = np.exp(lg - lg.max())
        attn = e / e.sum()
        bag_emb = attn @ emb[mask]
        out[bag] = bag_emb @ w_cls + b_cls
    return out


def kernel(**inputs):
    from concourse.bass_utils import run_bass_kernel_spmd

    import ml_dtypes

    x = np.asarray(inputs["x"], dtype=np.float32)
    seg = np.asarray(inputs["seg"], dtype=np.int32)
    w_enc = np.asarray(inputs["w_enc"], dtype=np.float32)
    b_enc = np.asarray(inputs["b_enc"], dtype=np.float32)
    w_att = np.asarray(inputs["w_att"], dtype=np.float32)
    b_att = np.asarray(inputs["b_att"], dtype=np.float32)
    w_score = np.asarray(inputs["w_score"], dtype=np.float32)
    b_score = np.asarray(inputs["b_score"], dtype=np.float32)
    w_cls = np.asarray(inputs["w_cls"], dtype=np.float32)
    b_cls = np.asarray(inputs["b_cls"], dtype=np.float32)

    expected_seg = np.repeat(np.arange(N_BAGS, dtype=np.int32), INST_PER_BAG)
    if not np.array_equal(seg, expected_seg):
        # Layout differs from the balanced bags this kernel is built for.
        return _numpy_fallback(x, seg, w_enc, b_enc, w_att, b_att, w_score,
                               b_score, w_cls, b_cls)

    if "nc" not in _CACHE:
        _CACHE["nc"] = _build()
    nc = _CACHE["nc"]

    shared = {
        "w_enc": np.ascontiguousarray(
            w_enc.astype(ml_dtypes.bfloat16).reshape(DIN_CHUNKS, 128, D_EMB)
            .transpose(1, 0, 2).reshape(128, D_IN)),
        "b_enc": b_enc,
        "w_att": w_att.astype(ml_dtypes.bfloat16), "b_att": b_att,
        "w_score": w_score.astype(ml_dtypes.bfloat16),
        "w_cls": w_cls, "b_cls": b_cls,
    }
    in_maps = []
    for c in range(N_CORES):
        xs = x[c * INST_PER_CORE:(c + 1) * INST_PER_CORE]
        in_maps.append(
            {"xt": np.ascontiguousarray(xs.T).astype(ml_dtypes.bfloat16),
             **shared})

    res = run_bass_kernel_spmd(nc, in_maps, core_ids=list(range(N_CORES)))
    return np.concatenate(
        [res.results[c]["out"].T for c in range(N_CORES)], axis=0)



# revision 12
# speedup vs baseline: 1.1123x; 1.1123x over previous
"""AttentionMIL Trainium2 kernel (fp8 encoder, v2).

Math (per bag of 512 instances):
    emb    = relu(x @ w_enc + b_enc)            [512, 128]
    a      = tanh(emb @ w_att + b_att)          [512, 64]
    logits = a @ w_score (+ b_score, dropped: softmax shift-invariant)
    attn   = softmax(logits) within the bag
    bag    = sum_i attn[i] * emb[i]             [128]
    score  = bag @ w_cls + b_cls                [2]

Distribution: data-parallel over bags. 8 NeuronCores, 8 bags (4096
instances) per core, weights replicated, no cross-core communication.
Each core returns its 8 bags' scores transposed [2, 8]; host stacks.

v2 changes vs the bf16 baseline (50.8 us):
 - x and w_enc are fp8 e4m3 (final rel err ~6e-3 vs the 2e-2 gate,
   verified numerically on the reference). Halves HBM traffic again:
   4.2 MB/core, ~12 us at the ~358 GB/s HBM-per-core limit.
 - x arrives in 4 DMA pieces of 1.05 MB with 8 KB contiguous per
   partition line (the old layout produced 8592 1-KB descriptors that
   capped the HWDGE ring at ~180 GB/s; 128 8-KB descriptors per piece
   run at line rate).
 - Encoder matmuls use DoubleRow perf mode: lhsT [128, 2, 128] fp8
   packs two k-tiles per PE cell, contracting K=256 per instruction;
   4 instructions per bag instead of 8.
 - Attention tail processed per bag PAIR with array-tiled concurrent
   matmuls: a^T for two bags lands in one PSUM bank via col-split
   (tile_position (0,0)/(0,64)); logits via zero-padded w_score
   [64, 32] into quadrants (0,0)/(64,32) so one Exp call (with
   accum_out producing the softmax denominator for free) covers both
   bags; tanh also covers both bags per call. Halves ACT time.
 - Normalization folded into the broadcast matmul: lhsT is a row of
   1/denom (instead of ones), so the e-broadcast is already attn and
   the fused DVE tensor_tensor_reduce emits the normalized bag
   embedding directly. The epilogue is one f32 matmul + bias + DMA.
"""

import sys

sys.path.insert(0, "/opt/trn_rl_repo")

import numpy as np

N_INST = 32768
N_BAGS = 64
D_IN = 1024
D_EMB = 128
D_ATT = 64
N_CLS = 2

N_CORES = 8
BAGS_PER_CORE = N_BAGS // N_CORES          # 8
INST_PER_BAG = N_INST // N_BAGS            # 512
INST_PER_CORE = N_INST // N_CORES          # 4096
PAIRS = BAGS_PER_CORE // 2                 # 4
DC = 4                                     # double-chunks of K=256
USE_DOUBLEROW = True                       # via DoubleRowSwInterleave: plain
                                           # DoubleRow kills the exec unit on
                                           # this toolchain (probe P1), but the
                                           # SW-interleaved variant works (P1b)
                                           # and halves encoder PE time

_CACHE = {}


def _build():
    import concourse.bacc as bacc
    import concourse.mybir as mybir
    import concourse.tile as tile

    f32 = mybir.dt.float32
    f32r = mybir.dt.float32r
    bf16 = mybir.dt.bfloat16
    fp8 = mybir.dt.float8e4
    AF = mybir.ActivationFunctionType
    ALU = mybir.AluOpType
    DRS = mybir.MatmulPerfMode.DoubleRowSwInterleave

    nc = bacc.Bacc("TRN2", target_bir_lowering=False, debug=False,
                   enable_asserts=False, num_devices=N_CORES)

    # x packed on host: [piece, 128 part, 8 chunk * 1024 inst] fp8,
    # piece p = bags {2p, 2p+1}; partition = din % 128, chunk = din // 128
    xt = nc.dram_tensor("xt", [PAIRS, 128, 8 * 2 * INST_PER_BAG], fp8,
                        kind="ExternalInput")
    # w_enc packed on host: [128 part, dc, ktile, emb] flattened
    w8 = nc.dram_tensor("w8", [128, DC * 2 * D_EMB], fp8, kind="ExternalInput")
    b_enc = nc.dram_tensor("b_enc", [D_EMB], f32, kind="ExternalInput")
    watt = nc.dram_tensor("watt", [D_EMB, D_ATT], bf16, kind="ExternalInput")
    # b_att stacked twice (tanh covers a bag pair on 128 partitions)
    battp = nc.dram_tensor("battp", [128], f32, kind="ExternalInput")
    # w_score zero-padded to [128, 32]: rows 0-63 and 64-127 both hold
    # w_score in column 0, zeros elsewhere
    wspad = nc.dram_tensor("wspad", [128, 32], bf16, kind="ExternalInput")
    wcls = nc.dram_tensor("wcls", [D_EMB, N_CLS], f32, kind="ExternalInput")
    bcls = nc.dram_tensor("bcls", [N_CLS], f32, kind="ExternalInput")
    out = nc.dram_tensor("out", [N_CLS, BAGS_PER_CORE], f32,
                         kind="ExternalOutput")

    with tile.TileContext(nc) as tc:
        with (
            tc.tile_pool(name="const", bufs=1) as const,
            tc.tile_pool(name="xp", bufs=PAIRS) as xp_pool,
            tc.tile_pool(name="embp", bufs=4) as embp,
            tc.tile_pool(name="work", bufs=2) as work,
            tc.tile_pool(name="ps_emb", bufs=4, space="PSUM") as ps_emb,
            tc.tile_pool(name="ps_a", bufs=1, space="PSUM") as ps_a,
            tc.tile_pool(name="ps_l", bufs=1, space="PSUM") as ps_l,
            tc.tile_pool(name="ps_bc", bufs=2, space="PSUM") as ps_bc,
        ):
            # ---- replicated weights (scalar HWDGE queue, lands early) ----
            w8_sb = const.tile([128, DC, 2, D_EMB], fp8)
            nc.scalar.dma_start(
                out=w8_sb,
                in_=w8[:, :].rearrange("p (a b e) -> p a b e", a=DC, b=2))
            benc_sb = const.tile([D_EMB, 1], f32)
            nc.scalar.dma_start(
                out=benc_sb, in_=b_enc[:].rearrange("(p one) -> p one", one=1))
            watt_sb = const.tile([D_EMB, D_ATT], bf16)
            nc.scalar.dma_start(out=watt_sb, in_=watt[:, :])
            battp_sb = const.tile([128, 1], f32)
            nc.scalar.dma_start(
                out=battp_sb, in_=battp[:].rearrange("(p one) -> p one", one=1))
            wspad_sb = const.tile([128, 32], bf16)
            nc.scalar.dma_start(out=wspad_sb, in_=wspad[:, :])
            wcls_sb = const.tile([D_EMB, N_CLS], f32)
            nc.scalar.dma_start(out=wcls_sb, in_=wcls[:, :])
            bcls_sb = const.tile([N_CLS, 1], f32)
            nc.scalar.dma_start(
                out=bcls_sb, in_=bcls[:].rearrange("(p one) -> p one", one=1))
            ones_sb = const.tile([64, 64], bf16)
            nc.vector.memset(ones_sb, 1.0)

            # normalized bag embeddings, column per bag
            bag_all = const.tile([D_EMB, BAGS_PER_CORE], f32)

            # ---- x pieces: one 1.05 MB DMA per bag pair (sync queue) ----
            xpieces = []
            for p in range(PAIRS):
                xp = xp_pool.tile([128, 8, 2 * INST_PER_BAG], fp8, tag="xp")
                nc.sync.dma_start(
                    out=xp,
                    in_=xt[p, :, :].rearrange("p (c i) -> p c i", c=8))
                xpieces.append(xp)

            def emit_enc(p):
                # emb^T for both bags of pair p
                pse = [ps_emb.tile([D_EMB, INST_PER_BAG], f32, tag="emb",
                                   name=f"pse{p}_{j}")
                       for j in range(2)]
                xp = xpieces[p]
                if USE_DOUBLEROW:
                    # DoubleRow K=256 per MM (weights host-interleaved)
                    for dc in range(DC):
                        for j in range(2):
                            nc.tensor.matmul(
                                pse[j][:, :], w8_sb[:, dc, :, :],
                                xp[:, 2 * dc:2 * dc + 2,
                                   j * INST_PER_BAG:(j + 1) * INST_PER_BAG],
                                start=(dc == 0), stop=(dc == DC - 1),
                                perf_mode=DRS)
                else:
                    for c in range(8):
                        for j in range(2):
                            nc.tensor.matmul(
                                pse[j][:, :], w8_sb[:, c // 2, c % 2, :],
                                xp[:, c,
                                   j * INST_PER_BAG:(j + 1) * INST_PER_BAG],
                                start=(c == 0), stop=(c == 7))
                return pse

            def emit_relu(pse):
                embs = []
                for j in range(2):
                    e = embp.tile([D_EMB, INST_PER_BAG], bf16, tag="embT")
                    nc.scalar.activation(e, pse[j], AF.Relu, bias=benc_sb,
                                         scale=1.0)
                    embs.append(e)
                return embs

            def emit_pair_head(embs, sl, e_t, den_col):
                """att+tanh+score+exp for both bags over instance slice sl.

                Writes exp(logits) rows into e_t (row 0 = even bag, row 32 =
                odd bag) and per-row sums into den_col [64, 1].
                """
                n = sl.stop - sl.start
                # a^T pair: col-split quadrants of one PSUM bank
                ps_a_t = ps_a.tile([128, INST_PER_BAG], f32, tag="a")
                nc.tensor.matmul(ps_a_t[0:64, :n], watt_sb[:, :], embs[0][:, sl],
                                 start=True, stop=True, tile_position=(0, 0))
                nc.tensor.matmul(ps_a_t[64:128, :n], watt_sb[:, :], embs[1][:, sl],
                                 start=True, stop=True, tile_position=(0, 64))
                aT = work.tile([128, INST_PER_BAG], bf16, tag="aT")
                nc.scalar.activation(aT[:, :n], ps_a_t[:, :n], AF.Tanh,
                                     bias=battp_sb, scale=1.0)
                # logits: zero-padded w_score into two disjoint PE quadrants;
                # row 0 = even bag logits, row 32 = odd bag, rest zeros
                ps_l_t = ps_l.tile([64, INST_PER_BAG], f32, tag="logit")
                nc.tensor.matmul(ps_l_t[0:32, :n], wspad_sb[0:64, :],
                                 aT[0:64, 0:n],
                                 start=True, stop=True, tile_position=(0, 0))
                nc.tensor.matmul(ps_l_t[32:64, :n], wspad_sb[64:128, :],
                                 aT[64:128, 0:n],
                                 start=True, stop=True, tile_position=(64, 32))
                # exp with free softmax denominator (rows 1-31/33-63 hold
                # exp(0)=1 from the zero padding; harmless, never read).
                # No max-shift: |logits| <= ||w_score||_1 ~ 6, exp is safe.
                nc.scalar.activation(e_t[:, sl], ps_l_t[:, :n], AF.Exp,
                                     scale=1.0, accum_out=den_col)

            def emit_pair_norm_tail(p, embs, e_t, den_col):
                """1/denom -> normalized broadcast -> bag embeddings."""
                rv = work.tile([64, 1], f32, tag="rv")
                nc.vector.reciprocal(rv, den_col)
                rvrow = work.tile([64, 64], bf16, tag="rvrow")
                nc.vector.tensor_scalar_mul(rvrow, ones_sb[:, :], rv)
                for j in range(2):
                    r0 = 32 * j
                    b = 2 * p + j
                    ps_bc_t = ps_bc.tile([D_EMB, INST_PER_BAG], f32, tag="bc")
                    nc.tensor.matmul(ps_bc_t[0:64, :], rvrow[r0:r0 + 1, :],
                                     e_t[r0:r0 + 1, :], start=True, stop=True,
                                     tile_position=(r0, 0))
                    nc.tensor.matmul(ps_bc_t[64:128, :], rvrow[r0:r0 + 1, :],
                                     e_t[r0:r0 + 1, :], start=True, stop=True,
                                     tile_position=(r0, 64))
                    # tensor_tensor_reduce would fuse these, but it kills the
                    # exec unit on this toolchain (probe P5c)
                    scratch = work.tile([D_EMB, INST_PER_BAG], bf16,
                                        tag="scratch")
                    nc.vector.tensor_mul(scratch, embs[j], ps_bc_t)
                    nc.vector.reduce_sum(bag_all[:, b:b + 1], scratch,
                                         axis=mybir.AxisListType.X)

            def emit_tail(p, embs):
                e_t = work.tile([64, INST_PER_BAG], bf16, tag="e")
                den_col = work.tile([64, 1], f32, tag="den")
                emit_pair_head(embs, slice(0, INST_PER_BAG), e_t, den_col)
                emit_pair_norm_tail(p, embs, e_t, den_col)

            def emit_tail_halved(p, embs):
                # last pair: half-split the head so PE/ACT stages pipeline
                # against each other on the end-of-kernel serial chain
                H = INST_PER_BAG // 2
                e_t = work.tile([64, INST_PER_BAG], bf16, tag="e")
                den_h = work.tile([64, 2], f32, tag="den_h")
                for h in range(2):
                    emit_pair_head(embs, slice(h * H, (h + 1) * H), e_t,
                                   den_h[:, h:h + 1])
                den_col = work.tile([64, 1], f32, tag="den")
                nc.vector.tensor_add(den_col, den_h[:, 0:1], den_h[:, 1:2])
                emit_pair_norm_tail(p, embs, e_t, den_col)

            # software pipeline: pair p's tail emitted after pair p+1's
            # encoder matmuls so the in-order PE queue never head-of-line
            # blocks on the softmax chain
            prev = None
            for p in range(PAIRS):
                pse = emit_enc(p)
                if prev is not None:
                    emit_tail(p - 1, prev)
                prev = emit_relu(pse)
            emit_tail_halved(PAIRS - 1, prev)

            # scores^T = w_cls.T @ bag + b_cls   [2, 8]
            ps_s = ps_l.tile([N_CLS, BAGS_PER_CORE], f32, tag="logit")
            nc.tensor.matmul(ps_s[:, :], wcls_sb[:, :], bag_all[:, :],
                             start=True, stop=True)
            scores = work.tile([N_CLS, BAGS_PER_CORE], f32, tag="scores")
            nc.scalar.activation(scores, ps_s, AF.Identity, bias=bcls_sb,
                                 scale=1.0)
            nc.scalar.dma_start(out=out[:, :], in_=scores)

    nc.compile()
    return nc


def prep_in_maps(inputs):
    """Pack full-size inputs into the per-core input maps."""
    import ml_dtypes

    fp8 = ml_dtypes.float8_e4m3
    bf16 = ml_dtypes.bfloat16

    x = np.asarray(inputs["x"], dtype=np.float32)
    w_enc = np.asarray(inputs["w_enc"], dtype=np.float32)
    b_enc = np.asarray(inputs["b_enc"], dtype=np.float32)
    w_att = np.asarray(inputs["w_att"], dtype=np.float32)
    b_att = np.asarray(inputs["b_att"], dtype=np.float32)
    w_score = np.asarray(inputs["w_score"], dtype=np.float32)
    w_cls = np.asarray(inputs["w_cls"], dtype=np.float32)
    b_cls = np.asarray(inputs["b_cls"], dtype=np.float32)

    # w_enc [1024, 128] -> [p, dc, ktile, emb]: din = dc*256 + ktile*128 + p
    w_dr = (w_enc.reshape(DC, 2, 128, D_EMB).transpose(2, 0, 1, 3)
            .astype(fp8))                  # [p, dc, ktile, emb]
    if USE_DOUBLEROW:
        # DoubleRowSwInterleave HW layout per dc:
        #   w_hw[p, 2*(127-m) + i] = w_dr[p, i, m]
        w8 = np.zeros((128, DC, 2 * D_EMB), dtype=fp8)
        w8[:, :, 0::2] = w_dr[:, :, 0, ::-1]
        w8[:, :, 1::2] = w_dr[:, :, 1, ::-1]
        w8 = np.ascontiguousarray(w8.reshape(128, DC * 2 * D_EMB))
    else:
        w8 = np.ascontiguousarray(w_dr.reshape(128, DC * 2 * D_EMB))
    wspad = np.zeros((128, 32), dtype=bf16)
    wspad[0:64, 0] = w_score.astype(bf16)
    wspad[64:128, 0] = w_score.astype(bf16)
    battp = np.concatenate([b_att, b_att])

    shared = {
        "w8": w8,
        "b_enc": b_enc,
        "watt": w_att.astype(bf16),
        "battp": battp,
        "wspad": wspad,
        "wcls": w_cls,
        "bcls": b_cls,
    }
    in_maps = []
    for c in range(N_CORES):
        xs = x[c * INST_PER_CORE:(c + 1) * INST_PER_CORE]
        # [4096, 1024] -> T -> [chunk, p, pair, inst] -> [pair, p, chunk, inst]
        xtc = np.ascontiguousarray(
            xs.T.reshape(8, 128, PAIRS, 2 * INST_PER_BAG)
            .transpose(2, 1, 0, 3)
            .reshape(PAIRS, 128, 8 * 2 * INST_PER_BAG)).astype(fp8)
        in_maps.append({"xt": xtc, **shared})
    return in_maps


def _numpy_fallback(x, seg, w_enc, b_enc, w_att, b_att, w_score, b_score,
                    w_cls, b_cls):
    emb = np.maximum(x @ w_enc + b_enc, 0.0)
    a = np.tanh(emb @ w_att + b_att)
    logits = a @ w_score + b_score[0]
    out = np.zeros((N_BAGS, N_CLS), dtype=np.float32)
    for bag in range(N_BAGS):
        mask = seg == bag
        lg = logits[mask]
        e = np.exp(lg - lg.max())
        attn = e / e.sum()
        bag_emb = attn @ emb[mask]
        out[bag] = bag_emb @ w_cls + b_cls
    return out


def kernel(**inputs):
    from concourse.bass_utils import run_bass_kernel_spmd

    seg = np.asarray(inputs["seg"], dtype=np.int32)
    expected_seg = np.repeat(np.arange(N_BAGS, dtype=np.int32), INST_PER_BAG)
    if not np.array_equal(seg, expected_seg):
        # Layout differs from the balanced bags this kernel is built for.
        return _numpy_fallback(
            np.asarray(inputs["x"], dtype=np.float32), seg,
            *(np.asarray(inputs[k], dtype=np.float32) for k in
              ("w_enc", "b_enc", "w_att", "b_att", "w_score", "b_score",
               "w_cls", "b_cls")))

    if "nc" not in _CACHE:
        _CACHE["nc"] = _build()
    nc = _CACHE["nc"]

    in_maps = prep_in_maps(inputs)
    res = run_bass_kernel_spmd(nc, in_maps, core_ids=list(range(N_CORES)))
    return np.concatenate(
        [res.results[c]["out"].T for c in range(N_CORES)], axis=0)


# revision 21
# speedup vs baseline: 1.1597x; 1.0426x over previous
"""AttentionMIL Trainium2 kernel (fp8 encoder, v2).

Math (per bag of 512 instances):
    emb    = relu(x @ w_enc + b_enc)            [512, 128]
    a      = tanh(emb @ w_att + b_att)          [512, 64]
    logits = a @ w_score (+ b_score, dropped: softmax shift-invariant)
    attn   = softmax(logits) within the bag
    bag    = sum_i attn[i] * emb[i]             [128]
    score  = bag @ w_cls + b_cls                [2]

Distribution: data-parallel over bags. 8 NeuronCores, 8 bags (4096
instances) per core, weights replicated, no cross-core communication.
Each core returns its 8 bags' scores transposed [2, 8]; host stacks.

v2 changes vs the bf16 baseline (50.8 us):
 - x and w_enc are fp8 e4m3 (final rel err ~6e-3 vs the 2e-2 gate,
   verified numerically on the reference). Halves HBM traffic again:
   4.2 MB/core, ~12 us at the ~358 GB/s HBM-per-core limit.
 - x arrives in 4 DMA pieces of 1.05 MB with 8 KB contiguous per
   partition line (the old layout produced 8592 1-KB descriptors that
   capped the HWDGE ring at ~180 GB/s; 128 8-KB descriptors per piece
   run at line rate).
 - Encoder matmuls use DoubleRow perf mode: lhsT [128, 2, 128] fp8
   packs two k-tiles per PE cell, contracting K=256 per instruction;
   4 instructions per bag instead of 8.
 - Attention tail processed per bag PAIR with array-tiled concurrent
   matmuls: a^T for two bags lands in one PSUM bank via col-split
   (tile_position (0,0)/(0,64)); logits via zero-padded w_score
   [64, 32] into quadrants (0,0)/(64,32) so one Exp call (with
   accum_out producing the softmax denominator for free) covers both
   bags; tanh also covers both bags per call. Halves ACT time.
 - Normalization folded into the broadcast matmul: lhsT is a row of
   1/denom (instead of ones), so the e-broadcast is already attn and
   the fused DVE tensor_tensor_reduce emits the normalized bag
   embedding directly. The epilogue is one f32 matmul + bias + DMA.
"""

import sys

sys.path.insert(0, "/opt/trn_rl_repo")

import numpy as np

N_INST = 32768
N_BAGS = 64
D_IN = 1024
D_EMB = 128
D_ATT = 64
N_CLS = 2

N_CORES = 8
BAGS_PER_CORE = N_BAGS // N_CORES          # 8
INST_PER_BAG = N_INST // N_BAGS            # 512
INST_PER_CORE = N_INST // N_CORES          # 4096
PAIRS = BAGS_PER_CORE // 2                 # 4
DC = 4                                     # double-chunks of K=256
USE_DOUBLEROW = True                       # via DoubleRowSwInterleave: plain
                                           # DoubleRow kills the exec unit on
                                           # this toolchain (probe P1), but the
                                           # SW-interleaved variant works (P1b)
                                           # and halves encoder PE time

_CACHE = {}


def _build():
    import concourse.bacc as bacc
    import concourse.mybir as mybir
    import concourse.tile as tile

    f32 = mybir.dt.float32
    f32r = mybir.dt.float32r
    bf16 = mybir.dt.bfloat16
    fp8 = mybir.dt.float8e4
    AF = mybir.ActivationFunctionType
    ALU = mybir.AluOpType
    DRS = mybir.MatmulPerfMode.DoubleRowSwInterleave

    nc = bacc.Bacc("TRN2", target_bir_lowering=False, debug=False,
                   enable_asserts=False, num_devices=N_CORES)

    # x packed on host: [bag, 128 part, 8 chunk * 512 inst] fp8,
    # partition = din % 128, chunk = din // 128; 4 KB contiguous per
    # partition per piece
    xt = nc.dram_tensor("xt", [BAGS_PER_CORE, 128, 8 * INST_PER_BAG], fp8,
                        kind="ExternalInput")
    # w_enc packed on host: [128 part, dc, ktile, emb] flattened and
    # DoubleRowSwInterleave-interleaved
    w8 = nc.dram_tensor("w8", [128, DC * 2 * D_EMB], fp8, kind="ExternalInput")
    # all small bf16 weights in one blob: watt | wspad | ones
    wbf = nc.dram_tensor("wbf", [128, 160], bf16, kind="ExternalInput")
    # all small f32 weights in one blob: benc | battp | wcls | bcls | zero
    wf32 = nc.dram_tensor("wf32", [128, 8], f32, kind="ExternalInput")
    out = nc.dram_tensor("out", [N_CLS, BAGS_PER_CORE], f32,
                         kind="ExternalOutput")

    with tile.TileContext(nc) as tc:
        with (
            tc.tile_pool(name="const", bufs=1) as const,
            tc.tile_pool(name="xp", bufs=BAGS_PER_CORE) as xp_pool,
            tc.tile_pool(name="embp", bufs=4) as embp,
            tc.tile_pool(name="work", bufs=2) as work,
            tc.tile_pool(name="ps_emb", bufs=3, space="PSUM") as ps_emb,
            tc.tile_pool(name="ps_a", bufs=1, space="PSUM") as ps_a,
            tc.tile_pool(name="ps_l", bufs=1, space="PSUM") as ps_l,
            tc.tile_pool(name="ps_bc", bufs=2, space="PSUM") as ps_bc,
            tc.tile_pool(name="ps_cls", bufs=1, space="PSUM") as ps_cls,
        ):
            # ---- replicated weights: 3 DMAs on the scalar HWDGE queue,
            # w8 FIRST (the first encoder matmul blocks on it) ----
            w8_sb = const.tile([128, DC, 2, D_EMB], fp8)
            nc.scalar.dma_start(
                out=w8_sb,
                in_=w8[:, :].rearrange("p (a b e) -> p a b e", a=DC, b=2))
            wbf_sb = const.tile([128, 160], bf16)
            nc.scalar.dma_start(out=wbf_sb, in_=wbf[:, :])
            wf32_sb = const.tile([128, 8], f32)
            nc.scalar.dma_start(out=wf32_sb, in_=wf32[:, :])

            watt_sb = wbf_sb[:, 0:64]
            wspad_sb = wbf_sb[:, 64:96]
            ones_sb = wbf_sb[:, 96:160]
            benc_sb = wf32_sb[:, 0:1]
            battp_sb = wf32_sb[:, 1:2]
            wcls_sb = wf32_sb[:, 2:4]
            bcls_sb = wf32_sb[0:N_CLS, 4:5]
            zerob_sb = wf32_sb[0:64, 5:6]

            # normalized bag embeddings, column per bag
            bag_all = const.tile([D_EMB, BAGS_PER_CORE], f32)

            # ---- x: one 0.52 MB DMA per bag (sync queue) ----
            xpieces = []
            for b in range(BAGS_PER_CORE):
                xp = xp_pool.tile([128, 8, INST_PER_BAG], fp8, tag="xp",
                                  name=f"xp{b}")
                nc.sync.dma_start(
                    out=xp,
                    in_=xt[b, :, :].rearrange("p (c i) -> p c i", c=8))
                xpieces.append(xp)

            def emit_enc(b):
                # emb^T for bag b
                pse = ps_emb.tile([D_EMB, INST_PER_BAG], f32, tag="emb",
                                  name=f"pse{b}")
                xp = xpieces[b]
                if USE_DOUBLEROW:
                    # DoubleRow K=256 per MM (weights host-interleaved)
                    for dc in range(DC):
                        nc.tensor.matmul(
                            pse[:, :], w8_sb[:, dc, :, :],
                            xp[:, 2 * dc:2 * dc + 2, :],
                            start=(dc == 0), stop=(dc == DC - 1),
                            perf_mode=DRS)
                else:
                    for c in range(8):
                        nc.tensor.matmul(
                            pse[:, :], w8_sb[:, c // 2, c % 2, :], xp[:, c, :],
                            start=(c == 0), stop=(c == 7))
                return pse

            def emit_relu(pse, b):
                e = embp.tile([D_EMB, INST_PER_BAG], bf16, tag="embT",
                              name=f"embT{b}")
                nc.scalar.activation(e, pse, AF.Relu, bias=benc_sb, scale=1.0)
                return e

            def emit_pair_head(embs, sl, e_t, den_col):
                """att+tanh+score+exp for both bags over instance slice sl.

                Writes exp(logits) rows into e_t (row 0 = even bag, row 32 =
                odd bag) and per-row sums into den_col [64, 1].
                """
                n = sl.stop - sl.start
                # a^T pair: col-split quadrants of one PSUM bank
                ps_a_t = ps_a.tile([128, INST_PER_BAG], f32, tag="a")
                nc.tensor.matmul(ps_a_t[0:64, :n], watt_sb[:, :], embs[0][:, sl],
                                 start=True, stop=True, tile_position=(0, 0))
                nc.tensor.matmul(ps_a_t[64:128, :n], watt_sb[:, :], embs[1][:, sl],
                                 start=True, stop=True, tile_position=(0, 64))
                aT = work.tile([128, INST_PER_BAG], bf16, tag="aT")
                nc.scalar.activation(aT[:, :n], ps_a_t[:, :n], AF.Tanh,
                                     bias=battp_sb, scale=1.0)
                # logits: zero-padded w_score into two disjoint PE quadrants;
                # row 0 = even bag logits, row 32 = odd bag, rest zeros
                ps_l_t = ps_l.tile([64, INST_PER_BAG], f32, tag="logit")
                nc.tensor.matmul(ps_l_t[0:32, :n], wspad_sb[0:64, :],
                                 aT[0:64, 0:n],
                                 start=True, stop=True, tile_position=(0, 0))
                nc.tensor.matmul(ps_l_t[32:64, :n], wspad_sb[64:128, :],
                                 aT[64:128, 0:n],
                                 start=True, stop=True, tile_position=(64, 32))
                # exp with free softmax denominator (rows 1-31/33-63 hold
                # exp(0)=1 from the zero padding; harmless, never read).
                # No max-shift: |logits| <= ||w_score||_1 ~ 6, exp is safe.
                nc.scalar.activation(e_t[:, sl], ps_l_t[:, :n], AF.Exp,
                                     bias=zerob_sb, scale=1.0,
                                     accum_out=den_col)

            def emit_pair_norm_tail(p, embs, e_t, den_col):
                """1/denom -> normalized broadcast -> bag embeddings."""
                rv = work.tile([64, 1], f32, tag="rv")
                nc.vector.reciprocal(rv, den_col)
                rvrow = work.tile([64, 64], bf16, tag="rvrow")
                nc.vector.tensor_scalar_mul(rvrow, ones_sb[0:64, :], rv)
                for j in range(2):
                    r0 = 32 * j
                    b = 2 * p + j
                    ps_bc_t = ps_bc.tile([D_EMB, INST_PER_BAG], f32, tag="bc")
                    nc.tensor.matmul(ps_bc_t[0:64, :], rvrow[r0:r0 + 1, :],
                                     e_t[r0:r0 + 1, :], start=True, stop=True,
                                     tile_position=(r0, 0))
                    nc.tensor.matmul(ps_bc_t[64:128, :], rvrow[r0:r0 + 1, :],
                                     e_t[r0:r0 + 1, :], start=True, stop=True,
                                     tile_position=(r0, 64))
                    # tensor_tensor_reduce would fuse these, but it kills the
                    # exec unit on this toolchain (probe P5c)
                    scratch = work.tile([D_EMB, INST_PER_BAG], bf16,
                                        tag="scratch")
                    nc.vector.tensor_mul(scratch, embs[j], ps_bc_t)
                    nc.vector.reduce_sum(bag_all[:, b:b + 1], scratch,
                                         axis=mybir.AxisListType.X)
                # classifier contribution for this pair, accumulated into the
                # shared [2, 8] PSUM tile so only bias+DMA remain at the end
                nc.tensor.matmul(ps_s[:, 2 * p:2 * p + 2], wcls_sb,
                                 bag_all[:, 2 * p:2 * p + 2],
                                 start=True, stop=True)

            def emit_tail(p, embs):
                e_t = work.tile([64, INST_PER_BAG], bf16, tag="e")
                den_col = work.tile([64, 1], f32, tag="den")
                emit_pair_head(embs, slice(0, INST_PER_BAG), e_t, den_col)
                emit_pair_norm_tail(p, embs, e_t, den_col)

            def emit_tail_halved(p, embs):
                # last pair: half-split the head so PE/ACT stages pipeline
                # against each other on the end-of-kernel serial chain
                H = INST_PER_BAG // 2
                e_t = work.tile([64, INST_PER_BAG], bf16, tag="e")
                den_h = work.tile([64, 2], f32, tag="den_h")
                for h in range(2):
                    emit_pair_head(embs, slice(h * H, (h + 1) * H), e_t,
                                   den_h[:, h:h + 1])
                den_col = work.tile([64, 1], f32, tag="den")
                nc.vector.tensor_add(den_col, den_h[:, 0:1], den_h[:, 1:2])
                emit_pair_norm_tail(p, embs, e_t, den_col)

            # per-pair classifier contributions accumulate here
            ps_s = ps_cls.tile([N_CLS, BAGS_PER_CORE], f32, tag="cls")

            # software pipeline: pair p's tail emitted after pair p+1's
            # first encoder so the in-order PE queue never head-of-line
            # blocks on the softmax chain
            prev = None
            for p in range(PAIRS):
                pse0 = emit_enc(2 * p)
                if prev is not None:
                    emit_tail(p - 1, prev)
                e0 = emit_relu(pse0, 2 * p)
                pse1 = emit_enc(2 * p + 1)
                e1 = emit_relu(pse1, 2 * p + 1)
                prev = [e0, e1]
            emit_tail_halved(PAIRS - 1, prev)

            # scores^T already in ps_s; just bias and store   [2, 8]
            scores = work.tile([N_CLS, BAGS_PER_CORE], f32, tag="scores")
            nc.scalar.activation(scores, ps_s, AF.Identity, bias=bcls_sb,
                                 scale=1.0)
            nc.scalar.dma_start(out=out[:, :], in_=scores)

    nc.compile()
    return nc


def prep_in_maps(inputs):
    """Pack full-size inputs into the per-core input maps."""
    import ml_dtypes

    fp8 = ml_dtypes.float8_e4m3
    bf16 = ml_dtypes.bfloat16

    x = np.asarray(inputs["x"], dtype=np.float32)
    w_enc = np.asarray(inputs["w_enc"], dtype=np.float32)
    b_enc = np.asarray(inputs["b_enc"], dtype=np.float32)
    w_att = np.asarray(inputs["w_att"], dtype=np.float32)
    b_att = np.asarray(inputs["b_att"], dtype=np.float32)
    w_score = np.asarray(inputs["w_score"], dtype=np.float32)
    w_cls = np.asarray(inputs["w_cls"], dtype=np.float32)
    b_cls = np.asarray(inputs["b_cls"], dtype=np.float32)

    # w_enc [1024, 128] -> [p, dc, ktile, emb]: din = dc*256 + ktile*128 + p
    w_dr = (w_enc.reshape(DC, 2, 128, D_EMB).transpose(2, 0, 1, 3)
            .astype(fp8))                  # [p, dc, ktile, emb]
    if USE_DOUBLEROW:
        # DoubleRowSwInterleave HW layout per dc:
        #   w_hw[p, 2*(127-m) + i] = w_dr[p, i, m]
        w8 = np.zeros((128, DC, 2 * D_EMB), dtype=fp8)
        w8[:, :, 0::2] = w_dr[:, :, 0, ::-1]
        w8[:, :, 1::2] = w_dr[:, :, 1, ::-1]
        w8 = np.ascontiguousarray(w8.reshape(128, DC * 2 * D_EMB))
    else:
        w8 = np.ascontiguousarray(w_dr.reshape(128, DC * 2 * D_EMB))
    # bf16 blob: watt | wspad | ones
    wbf = np.zeros((128, 160), dtype=bf16)
    wbf[:, 0:64] = w_att.astype(bf16)
    wbf[0:64, 64] = w_score.astype(bf16)
    wbf[64:128, 64] = w_score.astype(bf16)
    wbf[:, 96:160] = np.ones((128, 64), dtype=bf16)
    # f32 blob: benc | battp | wcls | bcls | zero
    wf32 = np.zeros((128, 8), dtype=np.float32)
    wf32[:, 0] = b_enc
    wf32[:, 1] = np.concatenate([b_att, b_att])
    wf32[:, 2:4] = w_cls
    wf32[0:N_CLS, 4] = b_cls

    shared = {"w8": w8, "wbf": wbf, "wf32": wf32}
    in_maps = []
    for c in range(N_CORES):
        xs = x[c * INST_PER_CORE:(c + 1) * INST_PER_CORE]
        # [4096, 1024] -> T -> [chunk, p, bag, inst] -> [bag, p, chunk, inst]
        xtc = np.ascontiguousarray(
            xs.T.reshape(8, 128, BAGS_PER_CORE, INST_PER_BAG)
            .transpose(2, 1, 0, 3)
            .reshape(BAGS_PER_CORE, 128, 8 * INST_PER_BAG)).astype(fp8)
        in_maps.append({"xt": xtc, **shared})
    return in_maps


def _numpy_fallback(x, seg, w_enc, b_enc, w_att, b_att, w_score, b_score,
                    w_cls, b_cls):
    emb = np.maximum(x @ w_enc + b_enc, 0.0)
    a = np.tanh(emb @ w_att + b_att)
    logits = a @ w_score + b_score[0]
    out = np.zeros((N_BAGS, N_CLS), dtype=np.float32)
    for bag in range(N_BAGS):
        mask = seg == bag
        lg = logits[mask]
        e = np.exp(lg - lg.max())
        attn = e / e.sum()
        bag_emb = attn @ emb[mask]
        out[bag] = bag_emb @ w_cls + b_cls
    return out


def kernel(**inputs):
    from concourse.bass_utils import run_bass_kernel_spmd

    seg = np.asarray(inputs["seg"], dtype=np.int32)
    expected_seg = np.repeat(np.arange(N_BAGS, dtype=np.int32), INST_PER_BAG)
    if not np.array_equal(seg, expected_seg):
        # Layout differs from the balanced bags this kernel is built for.
        return _numpy_fallback(
            np.asarray(inputs["x"], dtype=np.float32), seg,
            *(np.asarray(inputs[k], dtype=np.float32) for k in
              ("w_enc", "b_enc", "w_att", "b_att", "w_score", "b_score",
               "w_cls", "b_cls")))

    if "nc" not in _CACHE:
        _CACHE["nc"] = _build()
    nc = _CACHE["nc"]

    in_maps = prep_in_maps(inputs)
    res = run_bass_kernel_spmd(nc, in_maps, core_ids=list(range(N_CORES)))
    return np.concatenate(
        [res.results[c]["out"].T for c in range(N_CORES)], axis=0)
